# revision 5
# baseline (speedup 1.0000x reference)
"""Trainium2 Bass kernel for nn_CustomANFIS (N=4096, D=128, R=256, O=64).

Math (reference):
  memb[n,r,d]  = exp(-(x[n,d]-c[r,d])^2 / (2 s[r,d]^2))
  str[n,r]     = prod_d memb = exp(-q[n,r]),
                 q = sum_d x^2 * A - x * (c/s^2) ... quadratic form:
                 q[n,r] = sum_d x^2[n,d]*A[d,r] + sum_d x[n,d]*B[d,r] + G[r]
                 with A = 1/(2 s^2), B = -c/s^2, G = sum_d c^2/(2 s^2)
  den[n]       = sum_r str + 1e-8
  W[n,r,:]     = x[n,:] @ coeffs[r,:D,:] + coeffs[r,D,:]
  out          = softmax_j( (1/den) * sum_r str[n,r] * W[n,r,j] )

Device algorithm (data-parallel over N across 8 cores; all matmuls fp32r):
  1. strengths^T [r=256 (2 part-tiles), n=512] via 2 accumulating matmuls
     (lhsT=A/B tiles, rhs = X^T / (X^T)^2) + ACT exp with per-partition
     bias = -G.
  2. den column per n-tile via matmul(lhsT = sT-slice, rhs = ones).
  3. T[n, d, j] = sum_r sT[r,n] * Cflat[r, (d,j)]  -- 16 moving chunks of
     512 per n-tile, accumulated over the 2 r K-tiles in PSUM.
     Plus a bias chunk Tb[n,j] = sum_r sT[r,n]*Cb[r,j].
  4. prod[n, j, dslot] = X[n,d] * T[n,d,j] (DVE/GPSIMD elementwise with
     broadcast APs; bias slot copied in by ACT), then reduce over the 129
     dslots (DVE) -> acc[n,j].
  5. logits = acc * 1/(den), softmax over j via ACT exp + accum.
"""

import numpy as np

N, D, R, O = 4096, 128, 256, 64
NCORES = 8
NS = N // NCORES          # 512 rows per core
NT = NS // 128            # 4 n-tiles per core
RT = R // 128             # 2 r k-tiles
DJ = D * O                # 8192
CHUNK = 512
NCHUNK = DJ // CHUNK      # 16 chunks (8 d-slots x 64 j each)
DPC = CHUNK // O          # 8 d-slots per chunk
DSLOTS = D + 1            # 128 d products + 1 bias slot

# chunks (of NCHUNK per n-tile) whose elementwise multiply runs on GPSIMD
# (via an ACT fp32 PSUM->SBUF copy) instead of DVE reading PSUM directly.
GPS_CHUNKS = 10

_CACHE = {}


def _build():
    import concourse.bass as bass
    import concourse.tile as tile
    from concourse import bacc, mybir

    f32 = mybir.dt.float32
    f32r = mybir.dt.float32r
    AF = mybir.ActivationFunctionType
    ALU = mybir.AluOpType
    ts = bass.ts

    nc = bacc.Bacc(
        "TRN2", target_bir_lowering=False, debug=False, num_devices=NCORES
    )

    xt_d = nc.dram_tensor("xt", [D, NS], f32, kind="ExternalInput").ap()
    xn_d = nc.dram_tensor("xn", [128, NT * D], f32, kind="ExternalInput").ap()
    a_d = nc.dram_tensor("a_p", [D, R], f32, kind="ExternalInput").ap()
    b_d = nc.dram_tensor("b_p", [D, R], f32, kind="ExternalInput").ap()
    ng_d = nc.dram_tensor("negg", [128, RT], f32, kind="ExternalInput").ap()
    c_d = nc.dram_tensor("cflat", [RT, 128, DJ], f32, kind="ExternalInput").ap()
    cb_d = nc.dram_tensor("cbias", [128, RT * O], f32, kind="ExternalInput").ap()
    on_d = nc.dram_tensor("onesd", [128, 2], f32, kind="ExternalInput").ap()
    out_d = nc.dram_tensor("out", [NS, O], f32, kind="ExternalOutput").ap()

    def r32(ap):
        return ap if ap.dtype == f32r else ap.bitcast(f32r)

    with tile.TileContext(nc) as tc:
        from contextlib import ExitStack

        with ExitStack() as ctx:
            konst = ctx.enter_context(tc.tile_pool(name="konst", bufs=1))
            cw = ctx.enter_context(tc.tile_pool(name="cw", bufs=1))
            stp = ctx.enter_context(tc.tile_pool(name="stp", bufs=1))
            prodp = ctx.enter_context(tc.tile_pool(name="prodp", bufs=2))
            small = ctx.enter_context(tc.tile_pool(name="small", bufs=3))
            psum = ctx.enter_context(tc.tile_pool(name="psum", bufs=6, space="PSUM"))

            # ---- parameter / input loads
            a_sb = konst.tile([D, R], f32r)
            nc.sync.dma_start(a_sb[:], a_d.bitcast(f32r))
            b_sb = konst.tile([D, R], f32r)
            nc.sync.dma_start(b_sb[:], b_d.bitcast(f32r))
            ng_sb = konst.tile([128, RT], f32)
            nc.sync.dma_start(ng_sb[:], ng_d)
            xt_sb = konst.tile([D, NS], f32r)
            nc.sync.dma_start(xt_sb[:], xt_d.bitcast(f32r))
            ones = konst.tile([128, 2], f32r)
            nc.sync.dma_start(ones[:], on_d.bitcast(f32r))

            # x^2 transposed (written rounded-to-fp32r for the PE)
            x2t = konst.tile([D, NS], f32r)
            nc.vector.tensor_tensor(
                x2t[:], xt_sb[:].bitcast(f32), xt_sb[:].bitcast(f32), ALU.mult
            )

            # ---- strengths^T: [r-tile partitions, n free]
            st_tiles = []
            for rt in range(RT):
                sps = psum.tile([128, NS], f32, tag="bank", name=f"sps{rt}")
                nc.tensor.matmul(
                    sps[:], r32(a_sb[:, ts(rt, 128)]), r32(x2t[:]),
                    start=True, stop=False,
                )
                nc.tensor.matmul(
                    sps[:], r32(b_sb[:, ts(rt, 128)]), r32(xt_sb[:]),
                    start=False, stop=True,
                )
                st = stp.tile([128, NS], f32r, name=f"st{rt}")
                nc.scalar.activation(
                    st[:], sps[:], AF.Exp, bias=ng_sb[:, rt : rt + 1], scale=-1.0
                )
                st_tiles.append(st)

            # ---- big weights
            cb_sb = cw.tile([128, RT * O], f32r)
            nc.sync.dma_start(cb_sb[:], cb_d.bitcast(f32r))
            xn_sb = konst.tile([128, NT * D], f32)
            nc.sync.dma_start(xn_sb[:], xn_d)
            c_sb = cw.tile([128, RT * DJ], f32r)
            for rt in range(RT):
                for c in range(NCHUNK):
                    nc.sync.dma_start(
                        c_sb[:, rt * DJ + c * CHUNK : rt * DJ + (c + 1) * CHUNK],
                        c_d[rt, :, ts(c, CHUNK)].bitcast(f32r),
                    )

            # ---- per n-tile pipeline
            for nt in range(NT):
                st_n = [st[:, ts(nt, 128)] for st in st_tiles]

                # den column = sum_r strengths
                dps = psum.tile([128, CHUNK], f32, tag="bank", name=f"dps{nt}")
                nc.tensor.matmul(
                    dps[:, :2], r32(st_n[0]), r32(ones[:]), start=True, stop=False
                )
                nc.tensor.matmul(
                    dps[:, :2], r32(st_n[1]), r32(ones[:]), start=False, stop=True
                )
                denc = small.tile([128, 1], f32, name=f"denc{nt}")
                nc.vector.tensor_scalar_add(denc[:], dps[:, :1], 1e-8)
                scalec = small.tile([128, 1], f32, name=f"scalec{nt}")
                nc.vector.reciprocal(scalec[:], denc[:])

                # bias consequent Tb
                bps = psum.tile([128, CHUNK], f32, tag="bank", name=f"bps{nt}")
                nc.tensor.matmul(
                    bps[:, :O], r32(st_n[0]), r32(cb_sb[:, 0:O]),
                    start=True, stop=False,
                )
                nc.tensor.matmul(
                    bps[:, :O], r32(st_n[1]), r32(cb_sb[:, O : 2 * O]),
                    start=False, stop=True,
                )

                prod = prodp.tile([128, O, DSLOTS], f32, name=f"prod{nt}", tag="prod")
                # bias slot (ACT copies PSUM -> strided SBUF)
                nc.scalar.activation(prod[:, :, D], bps[:, :O], AF.Copy)

                xrow = xn_sb[:, ts(nt, D)]  # [128 n, 128 d]
                for c in range(NCHUNK):
                    tps = psum.tile([128, CHUNK], f32, tag="bank", name=f"tps{nt}_{c}")
                    nc.tensor.matmul(
                        tps[:],
                        r32(st_n[0]),
                        r32(c_sb[:, 0 * DJ + c * CHUNK : 0 * DJ + (c + 1) * CHUNK]),
                        start=True, stop=False,
                    )
                    nc.tensor.matmul(
                        tps[:],
                        r32(st_n[1]),
                        r32(c_sb[:, 1 * DJ + c * CHUNK : 1 * DJ + (c + 1) * CHUNK]),
                        start=False, stop=True,
                    )
                    tview = tps[:].rearrange("p (d j) -> p d j", d=DPC)
                    xb = (
                        xrow[:, c * DPC : (c + 1) * DPC]
                        .unsqueeze(2)
                        .broadcast_to([128, DPC, O])
                    )
                    # prod memory layout [p, j, dslot]; write view [p, d, j]
                    oview = prod[:, :, c * DPC : (c + 1) * DPC].transpose([0, 2, 1])
                    if c < NCHUNK - GPS_CHUNKS:
                        nc.vector.tensor_tensor(oview, tview, xb, ALU.mult)
                    else:
                        tcp = small.tile([128, CHUNK], f32, tag="tcp", name=f"tcp{nt}_{c}")
                        nc.scalar.activation(tcp[:], tps[:], AF.Copy)
                        tcpv = tcp[:].rearrange("p (d j) -> p d j", d=DPC)
                        nc.gpsimd.tensor_tensor(oview, tcpv, xb, ALU.mult)

                # reduce over dslots -> acc
                acc = small.tile([128, O], f32, name=f"acc{nt}")
                nc.vector.tensor_reduce(
                    acc[:], prod[:, :, :], axis=mybir.AxisListType.X, op=ALU.add
                )

                # logits = acc / den ; softmax over j
                logits = small.tile([128, O], f32, name=f"logits{nt}")
                nc.scalar.activation(logits[:], acc[:], AF.Copy, scale=scalec[:])
                negm = small.tile([128, 1], f32, name=f"negm{nt}")
                nc.vector.tensor_reduce(
                    negm[:], logits[:], axis=mybir.AxisListType.X, op=ALU.max,
                    negate=True,
                )
                exps = small.tile([128, O], f32, name=f"exps{nt}")
                sume = small.tile([128, 1], f32, name=f"sume{nt}")
                nc.scalar.activation(
                    exps[:], logits[:], AF.Exp, bias=negm[:], accum_out=sume[:]
                )
                rs = small.tile([128, 1], f32, name=f"rs{nt}")
                nc.vector.reciprocal(rs[:], sume[:])
                osb = small.tile([128, O], f32, name=f"osb{nt}")
                nc.scalar.activation(osb[:], exps[:], AF.Copy, scale=rs[:])
                nc.sync.dma_start(out_d[ts(nt, 128), :], osb[:])

    nc.compile()
    return nc


def _prep_inputs(X, centers, sigmas, coeffs):
    """Host-side sharding + layout transforms (numpy only)."""
    X = np.ascontiguousarray(X, dtype=np.float32)
    centers = np.asarray(centers, dtype=np.float32)
    sigmas = np.asarray(sigmas, dtype=np.float32)
    coeffs = np.asarray(coeffs, dtype=np.float32)

    inv2s2 = 1.0 / (2.0 * sigmas * sigmas)            # [R, D]
    A = np.ascontiguousarray(inv2s2.T)                # [D, R]
    B = np.ascontiguousarray((-centers / (sigmas * sigmas)).T)  # [D, R]
    G = (centers * centers * inv2s2).sum(axis=1)      # [R]
    negG = np.ascontiguousarray(-G.reshape(RT, 128).T)  # [128, RT]

    Cflat = np.ascontiguousarray(coeffs[:, :D, :]).reshape(R, DJ)
    Ck = np.ascontiguousarray(Cflat.reshape(RT, 128, DJ))  # [RT, 128, DJ]
    Cb = np.ascontiguousarray(
        coeffs[:, D, :].reshape(RT, 128, O).transpose(1, 0, 2).reshape(128, RT * O)
    )

    in_maps = []
    for i in range(NCORES):
        Xs = X[i * NS : (i + 1) * NS]                  # [512, 128]
        xt = np.ascontiguousarray(Xs.T)                # [128, 512]
        xn = np.ascontiguousarray(
            Xs.reshape(NT, 128, D).transpose(1, 0, 2).reshape(128, NT * D)
        )
        in_maps.append(
            {
                "xt": xt,
                "xn": xn,
                "a_p": A,
                "b_p": B,
                "negg": negG,
                "cflat": Ck,
                "cbias": Cb,
                "onesd": np.ones((128, 2), dtype=np.float32),
            }
        )
    return in_maps


def kernel(X, centers, sigmas, coeffs):
    from concourse.bass_utils import run_bass_kernel_spmd

    if "nc" not in _CACHE:
        _CACHE["nc"] = _build()
    nc = _CACHE["nc"]

    in_maps = _prep_inputs(X, centers, sigmas, coeffs)
    res = run_bass_kernel_spmd(nc, in_maps, list(range(NCORES)))
    out = np.concatenate([res.results[i]["out"] for i in range(NCORES)], axis=0)
    return out.astype(np.float32)


if __name__ == "__main__":
    rng = np.random.default_rng(0)
    X = rng.standard_normal((N, D), dtype=np.float32)
    centers = 0.5 * rng.standard_normal((R, D)).astype(np.float32)
    sigmas = (1.5 + rng.random((R, D))).astype(np.float32)
    coeffs = (0.02 * rng.standard_normal((R, D + 1, O))).astype(np.float32)
    out = kernel(X=X, centers=centers, sigmas=sigmas, coeffs=coeffs)
    print(out.shape, out.dtype, out.sum(axis=1)[:4])


# revision 6
# speedup vs baseline: 1.1863x; 1.1863x over previous
"""Trainium2 Bass kernel for nn_CustomANFIS (N=4096, D=128, R=256, O=64).

Math (reference):
  memb[n,r,d]  = exp(-(x[n,d]-c[r,d])^2 / (2 s[r,d]^2))
  str[n,r]     = prod_d memb = exp(-q[n,r]) with
                 q[n,r] = sum_d x^2[n,d]*A[d,r] + sum_d x[n,d]*B[d,r] + G[r],
                 A = 1/(2 s^2), B = -c/s^2, G = sum_d c^2/(2 s^2)
  den[n]       = sum_r str + 1e-8
  W[n,r,:]     = x[n,:] @ coeffs[r,:D,:] + coeffs[r,D,:]
  out          = softmax_j( (1/den) * sum_r str[n,r] * W[n,r,j] )

Device algorithm (data-parallel over N across 8 cores):
  1. strengths^T [r (2 part-tiles), n=512] via 2 fp32r accumulating matmuls
     + ACT exp (per-partition bias=-G), written as bf16.
  2. den column per n-tile via matmul(lhsT = sT-slice, rhs = ones).
  3. T[n, (j,d)] = sum_r sT[r,n] * C[r, (j,d)] in bf16 (16 chunks of 512 =
     4 j x 128 d per n-tile, PSUM-accumulated over the 2 r K-tiles), plus
     Tb[n,j] = sum_r sT[r,n]*Cb[r,j].
  4. prod[n, j, d] = X[n,d] * T[n,j,d]: ACT casts PSUM->SBUF bf16, DVE
     multiplies at 2x (or reads PSUM fp32 directly for some chunks, GPSIMD
     takes a share); then a bf16 tree-reduction over d and a fused
     (tree + Tb) add -> acc[n,j].
  5. logits = acc/den; softmax over j via ACT exp + accum_out.
"""

import numpy as np
import ml_dtypes

N, D, R, O = 4096, 128, 256, 64
NCORES = 8
NS = N // NCORES          # 512 rows per core
NT = NS // 128            # 4 n-tiles per core
RT = R // 128             # 2 r k-tiles
DJ = D * O                # 8192
CHUNK = 512
NCHUNK = DJ // CHUNK      # 16 chunks (4 j x 128 d each)
JPC = CHUNK // D          # 4 j per chunk

# per n-tile chunk split: the first FP32_DIRECT chunks are multiplied by DVE
# straight from PSUM (fp32, 1x); the rest are ACT-cast to bf16 SBUF and
# multiplied by DVE at 2x, except the last GPS_CHUNKS which go to GPSIMD.
FP32_DIRECT = 4
GPS_CHUNKS = 2

_CACHE = {}
BF16 = ml_dtypes.bfloat16


def _build():
    import concourse.bass as bass
    import concourse.tile as tile
    from concourse import bacc, mybir

    f32 = mybir.dt.float32
    f32r = mybir.dt.float32r
    bf16 = mybir.dt.bfloat16
    AF = mybir.ActivationFunctionType
    ALU = mybir.AluOpType
    ts = bass.ts

    nc = bacc.Bacc(
        "TRN2", target_bir_lowering=False, debug=False, num_devices=NCORES
    )

    xt_d = nc.dram_tensor("xt", [D, NS], f32, kind="ExternalInput").ap()
    xn_d = nc.dram_tensor("xn", [128, NT * D], bf16, kind="ExternalInput").ap()
    a_d = nc.dram_tensor("a_p", [D, R], f32, kind="ExternalInput").ap()
    b_d = nc.dram_tensor("b_p", [D, R], f32, kind="ExternalInput").ap()
    ng_d = nc.dram_tensor("negg", [128, RT], f32, kind="ExternalInput").ap()
    c_d = nc.dram_tensor("cflat", [RT, 128, DJ], bf16, kind="ExternalInput").ap()
    cb_d = nc.dram_tensor("cbias", [128, RT * O], bf16, kind="ExternalInput").ap()
    on_d = nc.dram_tensor("onesd", [128, 2], bf16, kind="ExternalInput").ap()
    out_d = nc.dram_tensor("out", [NS, O], f32, kind="ExternalOutput").ap()

    def r32(ap):
        return ap if ap.dtype == f32r else ap.bitcast(f32r)

    with tile.TileContext(nc) as tc:
        from contextlib import ExitStack

        with ExitStack() as ctx:
            konst = ctx.enter_context(tc.tile_pool(name="konst", bufs=1))
            cw = ctx.enter_context(tc.tile_pool(name="cw", bufs=1))
            stp = ctx.enter_context(tc.tile_pool(name="stp", bufs=1))
            prodp = ctx.enter_context(tc.tile_pool(name="prodp", bufs=2))
            small = ctx.enter_context(tc.tile_pool(name="small", bufs=3))
            psum = ctx.enter_context(tc.tile_pool(name="psum", bufs=6, space="PSUM"))

            # ---- parameter / input loads
            a_sb = konst.tile([D, R], f32r)
            nc.sync.dma_start(a_sb[:], a_d.bitcast(f32r))
            b_sb = konst.tile([D, R], f32r)
            nc.sync.dma_start(b_sb[:], b_d.bitcast(f32r))
            ng_sb = konst.tile([128, RT], f32)
            nc.sync.dma_start(ng_sb[:], ng_d)
            xt_sb = konst.tile([D, NS], f32r)
            nc.sync.dma_start(xt_sb[:], xt_d.bitcast(f32r))
            ones = konst.tile([128, 2], bf16)
            nc.sync.dma_start(ones[:], on_d)

            # x^2 transposed (written rounded-to-fp32r for the PE)
            x2t = konst.tile([D, NS], f32r)
            nc.vector.tensor_tensor(
                x2t[:], xt_sb[:].bitcast(f32), xt_sb[:].bitcast(f32), ALU.mult
            )

            # ---- strengths^T: [r-tile partitions, n free], bf16
            st_tiles = []
            for rt in range(RT):
                sps = psum.tile([128, NS], f32, tag="bank", name=f"sps{rt}")
                nc.tensor.matmul(
                    sps[:], r32(a_sb[:, ts(rt, 128)]), r32(x2t[:]),
                    start=True, stop=False,
                )
                nc.tensor.matmul(
                    sps[:], r32(b_sb[:, ts(rt, 128)]), r32(xt_sb[:]),
                    start=False, stop=True,
                )
                st = stp.tile([128, NS], bf16, name=f"st{rt}")
                nc.scalar.activation(
                    st[:], sps[:], AF.Exp, bias=ng_sb[:, rt : rt + 1], scale=-1.0
                )
                st_tiles.append(st)

            # ---- big weights
            cb_sb = cw.tile([128, RT * O], bf16)
            nc.sync.dma_start(cb_sb[:], cb_d)
            xn_sb = konst.tile([128, NT * D], bf16)
            nc.sync.dma_start(xn_sb[:], xn_d)
            c_sb = cw.tile([128, RT * DJ], bf16)
            for rt in range(RT):
                for c in range(NCHUNK):
                    nc.sync.dma_start(
                        c_sb[:, rt * DJ + c * CHUNK : rt * DJ + (c + 1) * CHUNK],
                        c_d[rt, :, ts(c, CHUNK)],
                    )

            # ---- per n-tile pipeline
            for nt in range(NT):
                st_n = [st[:, ts(nt, 128)] for st in st_tiles]

                # den column = sum_r strengths
                dps = psum.tile([128, CHUNK], f32, tag="bank", name=f"dps{nt}")
                nc.tensor.matmul(
                    dps[:, :2], st_n[0], ones[:], start=True, stop=False
                )
                nc.tensor.matmul(
                    dps[:, :2], st_n[1], ones[:], start=False, stop=True
                )
                denc = small.tile([128, 1], f32, name=f"denc{nt}")
                nc.vector.tensor_scalar_add(denc[:], dps[:, :1], 1e-8)
                scalec = small.tile([128, 1], f32, name=f"scalec{nt}")
                nc.vector.reciprocal(scalec[:], denc[:])

                # bias consequent Tb
                bps = psum.tile([128, CHUNK], f32, tag="bank", name=f"bps{nt}")
                nc.tensor.matmul(
                    bps[:, :O], st_n[0], cb_sb[:, 0:O], start=True, stop=False
                )
                nc.tensor.matmul(
                    bps[:, :O], st_n[1], cb_sb[:, O : 2 * O], start=False, stop=True
                )

                # prod layout: [n, j, d] (d contiguous)
                prod = prodp.tile([128, O, D], bf16, name=f"prod{nt}", tag="prod")

                xrow = xn_sb[:, ts(nt, D)]  # [128 n, 128 d] bf16
                xb = xrow.unsqueeze(1).broadcast_to([128, JPC, D])
                for c in range(NCHUNK):
                    tps = psum.tile([128, CHUNK], f32, tag="bank", name=f"tps{nt}_{c}")
                    nc.tensor.matmul(
                        tps[:],
                        st_n[0],
                        c_sb[:, 0 * DJ + c * CHUNK : 0 * DJ + (c + 1) * CHUNK],
                        start=True, stop=False,
                    )
                    nc.tensor.matmul(
                        tps[:],
                        st_n[1],
                        c_sb[:, 1 * DJ + c * CHUNK : 1 * DJ + (c + 1) * CHUNK],
                        start=False, stop=True,
                    )
                    tview = tps[:].rearrange("p (j d) -> p j d", j=JPC)
                    oview = prod[:, c * JPC : (c + 1) * JPC, :]  # [128, 4, 128]
                    if c < FP32_DIRECT:
                        nc.vector.tensor_tensor(oview, tview, xb, ALU.mult)
                    else:
                        tcp = small.tile(
                            [128, JPC, D], bf16, tag="tcp", name=f"tcp{nt}_{c}"
                        )
                        nc.scalar.activation(tcp[:], tps[:], AF.Copy)
                        if c < NCHUNK - GPS_CHUNKS:
                            nc.vector.tensor_tensor(oview, tcp[:], xb, ALU.mult)
                        else:
                            nc.gpsimd.tensor_tensor(oview, tcp[:], xb, ALU.mult)

                # tree-reduction over d (bf16, contiguous innermost)
                sbuf_s = small.tile([128, O, D // 2], bf16, tag="tree", name=f"s{nt}")
                nc.vector.tensor_tensor(
                    sbuf_s[:], prod[:, :, 0 : D // 2], prod[:, :, D // 2 : D], ALU.add
                )
                h = D // 2
                while h > 1:
                    h //= 2
                    nc.vector.tensor_tensor(
                        sbuf_s[:, :, 0:h], sbuf_s[:, :, 0:h],
                        sbuf_s[:, :, h : 2 * h], ALU.add,
                    )

                # acc = tree + Tb  (fused, reads Tb straight from PSUM)
                acc = small.tile([128, O], f32, name=f"acc{nt}")
                nc.vector.scalar_tensor_tensor(
                    acc[:], sbuf_s[:, :, 0], 1.0, bps[:, :O], ALU.mult, ALU.add
                )

                # logits = acc / den ; softmax over j
                logits = small.tile([128, O], f32, name=f"logits{nt}")
                nc.scalar.activation(logits[:], acc[:], AF.Copy, scale=scalec[:])
                negm = small.tile([128, 1], f32, name=f"negm{nt}")
                nc.vector.tensor_reduce(
                    negm[:], logits[:], axis=mybir.AxisListType.X, op=ALU.max,
                    negate=True,
                )
                exps = small.tile([128, O], f32, name=f"exps{nt}")
                sume = small.tile([128, 1], f32, name=f"sume{nt}")
                nc.scalar.activation(
                    exps[:], logits[:], AF.Exp, bias=negm[:], accum_out=sume[:]
                )
                rs = small.tile([128, 1], f32, name=f"rs{nt}")
                nc.vector.reciprocal(rs[:], sume[:])
                osb = small.tile([128, O], f32, name=f"osb{nt}")
                nc.scalar.activation(osb[:], exps[:], AF.Copy, scale=rs[:])
                nc.sync.dma_start(out_d[ts(nt, 128), :], osb[:])

    nc.compile()
    return nc


def _prep_inputs(X, centers, sigmas, coeffs):
    """Host-side sharding + layout transforms (numpy only)."""
    X = np.ascontiguousarray(X, dtype=np.float32)
    centers = np.asarray(centers, dtype=np.float32)
    sigmas = np.asarray(sigmas, dtype=np.float32)
    coeffs = np.asarray(coeffs, dtype=np.float32)

    inv2s2 = 1.0 / (2.0 * sigmas * sigmas)            # [R, D]
    A = np.ascontiguousarray(inv2s2.T)                # [D, R]
    B = np.ascontiguousarray((-centers / (sigmas * sigmas)).T)  # [D, R]
    G = (centers * centers * inv2s2).sum(axis=1)      # [R]
    negG = np.ascontiguousarray(-G.reshape(RT, 128).T)  # [128, RT]

    # C in [r, (j, d)] layout, bf16
    Cjd = np.ascontiguousarray(coeffs[:, :D, :].transpose(0, 2, 1))  # [R, O, D]
    Ck = np.ascontiguousarray(Cjd.reshape(RT, 128, DJ).astype(BF16))
    Cb = np.ascontiguousarray(
        coeffs[:, D, :].reshape(RT, 128, O).transpose(1, 0, 2).reshape(128, RT * O)
    ).astype(BF16)

    in_maps = []
    for i in range(NCORES):
        Xs = X[i * NS : (i + 1) * NS]                  # [512, 128]
        xt = np.ascontiguousarray(Xs.T)                # [128, 512]
        xn = np.ascontiguousarray(
            Xs.reshape(NT, 128, D).transpose(1, 0, 2).reshape(128, NT * D)
        ).astype(BF16)
        in_maps.append(
            {
                "xt": xt,
                "xn": xn,
                "a_p": A,
                "b_p": B,
                "negg": negG,
                "cflat": Ck,
                "cbias": Cb,
                "onesd": np.ones((128, 2), dtype=BF16),
            }
        )
    return in_maps


def kernel(X, centers, sigmas, coeffs):
    from concourse.bass_utils import run_bass_kernel_spmd

    if "nc" not in _CACHE:
        _CACHE["nc"] = _build()
    nc = _CACHE["nc"]

    in_maps = _prep_inputs(X, centers, sigmas, coeffs)
    res = run_bass_kernel_spmd(nc, in_maps, list(range(NCORES)))
    out = np.concatenate([res.results[i]["out"] for i in range(NCORES)], axis=0)
    return out.astype(np.float32)


if __name__ == "__main__":
    rng = np.random.default_rng(0)
    X = rng.standard_normal((N, D), dtype=np.float32)
    centers = 0.5 * rng.standard_normal((R, D)).astype(np.float32)
    sigmas = (1.5 + rng.random((R, D))).astype(np.float32)
    coeffs = (0.02 * rng.standard_normal((R, D + 1, O))).astype(np.float32)
    out = kernel(X=X, centers=centers, sigmas=sigmas, coeffs=coeffs)
    print(out.shape, out.dtype, out.sum(axis=1)[:4])


# revision 7
# speedup vs baseline: 1.2716x; 1.0719x over previous
"""Trainium2 Bass kernel for nn_CustomANFIS (N=4096, D=128, R=256, O=64).

Math (reference):
  memb[n,r,d]  = exp(-(x[n,d]-c[r,d])^2 / (2 s[r,d]^2))
  str[n,r]     = prod_d memb = exp(-q[n,r]) with
                 q[n,r] = sum_d x^2[n,d]*A[d,r] + sum_d x[n,d]*B[d,r] + G[r],
                 A = 1/(2 s^2), B = -c/s^2, G = sum_d c^2/(2 s^2)
  den[n]       = sum_r str + 1e-8
  W[n,r,:]     = x[n,:] @ coeffs[r,:D,:] + coeffs[r,D,:]
  out          = softmax_j( (1/den) * sum_r str[n,r] * W[n,r,j] )

Device algorithm (data-parallel over N across 8 cores):
  1. strengths^T [r (2 part-tiles), n=512] via 2 fp32r accumulating matmuls
     + ACT exp (per-partition bias=-G), written as bf16.
  2. den column per n-tile via matmul(lhsT = sT-slice, rhs = ones).
  3. T[n, (j,d)] = sum_r sT[r,n] * C[r, (j,d)] in bf16 (16 chunks of 512 =
     4 j x 128 d per n-tile, PSUM-accumulated over the 2 r K-tiles), plus
     Tb[n,j] = sum_r sT[r,n]*Cb[r,j].
  4. prod[n, j, d] = X[n,d] * T[n,j,d]: ACT casts PSUM->SBUF bf16, DVE
     multiplies at 2x (or reads PSUM fp32 directly for some chunks, GPSIMD
     takes a share); then a bf16 tree-reduction over d and a fused
     (tree + Tb) add -> acc[n,j].
  5. logits = acc/den; softmax over j via ACT exp + accum_out.
"""

import numpy as np
import ml_dtypes

N, D, R, O = 4096, 128, 256, 64
NCORES = 8
NS = N // NCORES          # 512 rows per core
NT = NS // 128            # 4 n-tiles per core
RT = R // 128             # 2 r k-tiles
DJ = D * O                # 8192
CHUNK = 512
NCHUNK = DJ // CHUNK      # 16 chunks (4 j x 128 d each)
JPC = CHUNK // D          # 4 j per chunk

# per n-tile chunk split: the first FP32_DIRECT chunks are multiplied by DVE
# straight from PSUM (fp32, 1x); the rest are ACT-cast to bf16 SBUF and
# multiplied by DVE at 2x, except the last GPS_CHUNKS which go to GPSIMD.
FP32_DIRECT = 2
GPS_CHUNKS = 0

_CACHE = {}
BF16 = ml_dtypes.bfloat16


def _build():
    import concourse.bass as bass
    import concourse.tile as tile
    from concourse import bacc, mybir

    f32 = mybir.dt.float32
    f32r = mybir.dt.float32r
    bf16 = mybir.dt.bfloat16
    AF = mybir.ActivationFunctionType
    ALU = mybir.AluOpType
    ts = bass.ts

    nc = bacc.Bacc(
        "TRN2", target_bir_lowering=False, debug=False, num_devices=NCORES
    )

    xt_d = nc.dram_tensor("xt", [D, NS], f32, kind="ExternalInput").ap()
    xn_d = nc.dram_tensor("xn", [128, NT * D], bf16, kind="ExternalInput").ap()
    a_d = nc.dram_tensor("a_p", [D, R], f32, kind="ExternalInput").ap()
    b_d = nc.dram_tensor("b_p", [D, R], f32, kind="ExternalInput").ap()
    ng_d = nc.dram_tensor("negg", [128, RT], f32, kind="ExternalInput").ap()
    c_d = nc.dram_tensor("cflat", [RT, 128, DJ], bf16, kind="ExternalInput").ap()
    cb_d = nc.dram_tensor("cbias", [128, RT * O], bf16, kind="ExternalInput").ap()
    on_d = nc.dram_tensor("onesd", [128, 2], bf16, kind="ExternalInput").ap()
    out_d = nc.dram_tensor("out", [NS, O], f32, kind="ExternalOutput").ap()

    def r32(ap):
        return ap if ap.dtype == f32r else ap.bitcast(f32r)

    with tile.TileContext(nc) as tc:
        from contextlib import ExitStack

        with ExitStack() as ctx:
            konst = ctx.enter_context(tc.tile_pool(name="konst", bufs=1))
            cw = ctx.enter_context(tc.tile_pool(name="cw", bufs=1))
            stp = ctx.enter_context(tc.tile_pool(name="stp", bufs=1))
            prodp = ctx.enter_context(tc.tile_pool(name="prodp", bufs=3))
            small = ctx.enter_context(tc.tile_pool(name="small", bufs=3))
            psum = ctx.enter_context(tc.tile_pool(name="psum", bufs=7, space="PSUM"))

            # ---- parameter / input loads
            a_sb = konst.tile([D, R], f32r)
            nc.sync.dma_start(a_sb[:], a_d.bitcast(f32r))
            b_sb = konst.tile([D, R], f32r)
            nc.sync.dma_start(b_sb[:], b_d.bitcast(f32r))
            ng_sb = konst.tile([128, RT], f32)
            nc.sync.dma_start(ng_sb[:], ng_d)
            xt_sb = konst.tile([D, NS], f32r)
            nc.sync.dma_start(xt_sb[:], xt_d.bitcast(f32r))
            ones = konst.tile([128, 2], bf16)
            nc.sync.dma_start(ones[:], on_d)

            # x^2 transposed (written rounded-to-fp32r for the PE)
            x2t = konst.tile([D, NS], f32r)
            nc.vector.tensor_tensor(
                x2t[:], xt_sb[:].bitcast(f32), xt_sb[:].bitcast(f32), ALU.mult
            )

            # ---- strengths^T: [r-tile partitions, n free], bf16
            st_tiles = []
            for rt in range(RT):
                sps = psum.tile([128, NS], f32, tag="bank", name=f"sps{rt}")
                nc.tensor.matmul(
                    sps[:], r32(a_sb[:, ts(rt, 128)]), r32(x2t[:]),
                    start=True, stop=False,
                )
                nc.tensor.matmul(
                    sps[:], r32(b_sb[:, ts(rt, 128)]), r32(xt_sb[:]),
                    start=False, stop=True,
                )
                st = stp.tile([128, NS], bf16, name=f"st{rt}")
                nc.scalar.activation(
                    st[:], sps[:], AF.Exp, bias=ng_sb[:, rt : rt + 1], scale=-1.0
                )
                st_tiles.append(st)

            # ---- big weights
            cb_sb = cw.tile([128, RT * O], bf16)
            nc.sync.dma_start(cb_sb[:], cb_d)
            xn_sb = konst.tile([128, NT * D], bf16)
            nc.sync.dma_start(xn_sb[:], xn_d)
            c_sb = cw.tile([128, RT * DJ], bf16)
            for rt in range(RT):
                for c in range(NCHUNK):
                    nc.sync.dma_start(
                        c_sb[:, rt * DJ + c * CHUNK : rt * DJ + (c + 1) * CHUNK],
                        c_d[rt, :, ts(c, CHUNK)],
                    )

            # ---- per n-tile pipeline
            for nt in range(NT):
                st_n = [st[:, ts(nt, 128)] for st in st_tiles]

                # den column = sum_r strengths
                dps = psum.tile([128, CHUNK], f32, tag="bank", name=f"dps{nt}")
                nc.tensor.matmul(
                    dps[:, :2], st_n[0], ones[:], start=True, stop=False
                )
                nc.tensor.matmul(
                    dps[:, :2], st_n[1], ones[:], start=False, stop=True
                )
                denc = small.tile([128, 1], f32, name=f"denc{nt}")
                nc.vector.tensor_scalar_add(denc[:], dps[:, :1], 1e-8)
                scalec = small.tile([128, 1], f32, name=f"scalec{nt}")
                nc.vector.reciprocal(scalec[:], denc[:])

                # bias consequent Tb
                bps = psum.tile([128, CHUNK], f32, tag="bank", name=f"bps{nt}")
                nc.tensor.matmul(
                    bps[:, :O], st_n[0], cb_sb[:, 0:O], start=True, stop=False
                )
                nc.tensor.matmul(
                    bps[:, :O], st_n[1], cb_sb[:, O : 2 * O], start=False, stop=True
                )

                # prod layout: [n, j, d] (d contiguous)
                prod = prodp.tile([128, O, D], bf16, name=f"prod{nt}", tag="prod")

                xrow = xn_sb[:, ts(nt, D)]  # [128 n, 128 d] bf16
                xb = xrow.unsqueeze(1).broadcast_to([128, JPC, D])
                for c in range(NCHUNK):
                    tps = psum.tile([128, CHUNK], f32, tag="bank", name=f"tps{nt}_{c}")
                    nc.tensor.matmul(
                        tps[:],
                        st_n[0],
                        c_sb[:, 0 * DJ + c * CHUNK : 0 * DJ + (c + 1) * CHUNK],
                        start=True, stop=False,
                    )
                    nc.tensor.matmul(
                        tps[:],
                        st_n[1],
                        c_sb[:, 1 * DJ + c * CHUNK : 1 * DJ + (c + 1) * CHUNK],
                        start=False, stop=True,
                    )
                    tview = tps[:].rearrange("p (j d) -> p j d", j=JPC)
                    oview = prod[:, c * JPC : (c + 1) * JPC, :]  # [128, 4, 128]
                    if c < FP32_DIRECT:
                        nc.vector.tensor_tensor(oview, tview, xb, ALU.mult)
                    else:
                        tcp = small.tile(
                            [128, JPC, D], bf16, tag="tcp", name=f"tcp{nt}_{c}",
                            bufs=6,
                        )
                        nc.scalar.activation(tcp[:], tps[:], AF.Copy)
                        if c < NCHUNK - GPS_CHUNKS:
                            nc.vector.tensor_tensor(oview, tcp[:], xb, ALU.mult)
                        else:
                            nc.gpsimd.tensor_tensor(oview, tcp[:], xb, ALU.mult)

                # tree-reduction over d (bf16, contiguous innermost)
                sbuf_s = small.tile([128, O, D // 2], bf16, tag="tree", name=f"s{nt}")
                nc.vector.tensor_tensor(
                    sbuf_s[:], prod[:, :, 0 : D // 2], prod[:, :, D // 2 : D], ALU.add
                )
                h = D // 2
                while h > 1:
                    h //= 2
                    nc.vector.tensor_tensor(
                        sbuf_s[:, :, 0:h], sbuf_s[:, :, 0:h],
                        sbuf_s[:, :, h : 2 * h], ALU.add,
                    )

                # acc = tree + Tb  (fused, reads Tb straight from PSUM)
                acc = small.tile([128, O], f32, name=f"acc{nt}")
                nc.vector.scalar_tensor_tensor(
                    acc[:], sbuf_s[:, :, 0], 1.0, bps[:, :O], ALU.mult, ALU.add
                )

                # softmax over j of logits = acc/den, fused:
                # exp(acc*scalec - max(acc)*scalec), max taken on unscaled acc
                negm = small.tile([128, 1], f32, name=f"negm{nt}")
                nc.vector.tensor_reduce(
                    negm[:], acc[:], axis=mybir.AxisListType.X, op=ALU.max,
                    negate=True,
                )
                negmb = small.tile([128, 1], f32, name=f"negmb{nt}")
                nc.vector.tensor_tensor(negmb[:], negm[:], scalec[:], ALU.mult)
                exps = small.tile([128, O], f32, name=f"exps{nt}")
                sume = small.tile([128, 1], f32, name=f"sume{nt}")
                nc.scalar.activation(
                    exps[:], acc[:], AF.Exp, bias=negmb[:], scale=scalec[:],
                    accum_out=sume[:],
                )
                rs = small.tile([128, 1], f32, name=f"rs{nt}")
                nc.vector.reciprocal(rs[:], sume[:])
                osb = small.tile([128, O], f32, name=f"osb{nt}")
                nc.scalar.activation(osb[:], exps[:], AF.Copy, scale=rs[:])
                nc.sync.dma_start(out_d[ts(nt, 128), :], osb[:])

    nc.compile()
    return nc


def _prep_inputs(X, centers, sigmas, coeffs):
    """Host-side sharding + layout transforms (numpy only)."""
    X = np.ascontiguousarray(X, dtype=np.float32)
    centers = np.asarray(centers, dtype=np.float32)
    sigmas = np.asarray(sigmas, dtype=np.float32)
    coeffs = np.asarray(coeffs, dtype=np.float32)

    inv2s2 = 1.0 / (2.0 * sigmas * sigmas)            # [R, D]
    A = np.ascontiguousarray(inv2s2.T)                # [D, R]
    B = np.ascontiguousarray((-centers / (sigmas * sigmas)).T)  # [D, R]
    G = (centers * centers * inv2s2).sum(axis=1)      # [R]
    negG = np.ascontiguousarray(-G.reshape(RT, 128).T)  # [128, RT]

    # C in [r, (j, d)] layout, bf16
    Cjd = np.ascontiguousarray(coeffs[:, :D, :].transpose(0, 2, 1))  # [R, O, D]
    Ck = np.ascontiguousarray(Cjd.reshape(RT, 128, DJ).astype(BF16))
    Cb = np.ascontiguousarray(
        coeffs[:, D, :].reshape(RT, 128, O).transpose(1, 0, 2).reshape(128, RT * O)
    ).astype(BF16)

    in_maps = []
    for i in range(NCORES):
        Xs = X[i * NS : (i + 1) * NS]                  # [512, 128]
        xt = np.ascontiguousarray(Xs.T)                # [128, 512]
        xn = np.ascontiguousarray(
            Xs.reshape(NT, 128, D).transpose(1, 0, 2).reshape(128, NT * D)
        ).astype(BF16)
        in_maps.append(
            {
                "xt": xt,
                "xn": xn,
                "a_p": A,
                "b_p": B,
                "negg": negG,
                "cflat": Ck,
                "cbias": Cb,
                "onesd": np.ones((128, 2), dtype=BF16),
            }
        )
    return in_maps


def kernel(X, centers, sigmas, coeffs):
    from concourse.bass_utils import run_bass_kernel_spmd

    if "nc" not in _CACHE:
        _CACHE["nc"] = _build()
    nc = _CACHE["nc"]

    in_maps = _prep_inputs(X, centers, sigmas, coeffs)
    res = run_bass_kernel_spmd(nc, in_maps, list(range(NCORES)))
    out = np.concatenate([res.results[i]["out"] for i in range(NCORES)], axis=0)
    return out.astype(np.float32)


if __name__ == "__main__":
    rng = np.random.default_rng(0)
    X = rng.standard_normal((N, D), dtype=np.float32)
    centers = 0.5 * rng.standard_normal((R, D)).astype(np.float32)
    sigmas = (1.5 + rng.random((R, D))).astype(np.float32)
    coeffs = (0.02 * rng.standard_normal((R, D + 1, O))).astype(np.float32)
    out = kernel(X=X, centers=centers, sigmas=sigmas, coeffs=coeffs)
    print(out.shape, out.dtype, out.sum(axis=1)[:4])


# revision 8
# speedup vs baseline: 1.3345x; 1.0495x over previous
"""Trainium2 Bass kernel for nn_CustomANFIS (N=4096, D=128, R=256, O=64).

Math (reference):
  memb[n,r,d]  = exp(-(x[n,d]-c[r,d])^2 / (2 s[r,d]^2))
  str[n,r]     = prod_d memb = exp(-q[n,r]) with
                 q[n,r] = sum_d x^2[n,d]*A[d,r] + sum_d x[n,d]*B[d,r] + G[r],
                 A = 1/(2 s^2), B = -c/s^2, G = sum_d c^2/(2 s^2)
  den[n]       = sum_r str + 1e-8
  W[n,r,:]     = x[n,:] @ coeffs[r,:D,:] + coeffs[r,D,:]
  out          = softmax_j( (1/den) * sum_r str[n,r] * W[n,r,j] )

Device algorithm (data-parallel over N across 8 cores):
  1. strengths^T [r (2 part-tiles), n=512] via 2 fp32r accumulating matmuls
     + ACT exp (per-partition bias=-G), written as bf16.
  2. den column per n-tile via matmul(lhsT = sT-slice, rhs = ones).
  3. T[n, (j,d)] = sum_r sT[r,n] * C[r, (j,d)] in bf16 (16 chunks of 512 =
     4 j x 128 d per n-tile, PSUM-accumulated over the 2 r K-tiles), plus
     Tb[n,j] = sum_r sT[r,n]*Cb[r,j].
  4. prod[n, j, d] = X[n,d] * T[n,j,d]: ACT casts PSUM->SBUF bf16, DVE
     multiplies at 2x (or reads PSUM fp32 directly for some chunks, GPSIMD
     takes a share); then a bf16 tree-reduction over d and a fused
     (tree + Tb) add -> acc[n,j].
  5. logits = acc/den; softmax over j via ACT exp + accum_out.
"""

import numpy as np
import ml_dtypes

N, D, R, O = 4096, 128, 256, 64
NCORES = 8
NS = N // NCORES          # 512 rows per core
NT = NS // 128            # 4 n-tiles per core
RT = R // 128             # 2 r k-tiles
DJ = D * O                # 8192
CHUNK = 1024              # 2 PSUM banks per chunk
NCHUNK = DJ // CHUNK      # 8 chunks (8 j x 128 d each)
JPC = CHUNK // D          # 8 j per chunk
MM = 512                  # moving free dim per matmul

# per n-tile chunk split: the first FP32_DIRECT chunks are multiplied by DVE
# straight from PSUM (fp32, 1x); the rest are ACT-cast to bf16 SBUF and
# multiplied by DVE at 2x, except the last GPS_CHUNKS which go to GPSIMD.
FP32_DIRECT = 1
GPS_CHUNKS = 0

_CACHE = {}
BF16 = ml_dtypes.bfloat16


def _build():
    import concourse.bass as bass
    import concourse.tile as tile
    from concourse import bacc, mybir

    f32 = mybir.dt.float32
    f32r = mybir.dt.float32r
    bf16 = mybir.dt.bfloat16
    AF = mybir.ActivationFunctionType
    ALU = mybir.AluOpType
    ts = bass.ts

    nc = bacc.Bacc(
        "TRN2", target_bir_lowering=False, debug=False, num_devices=NCORES
    )

    xt_d = nc.dram_tensor("xt", [D, NS], f32, kind="ExternalInput").ap()
    xn_d = nc.dram_tensor("xn", [128, NT * D], bf16, kind="ExternalInput").ap()
    a_d = nc.dram_tensor("a_p", [D, R], f32, kind="ExternalInput").ap()
    b_d = nc.dram_tensor("b_p", [D, R], f32, kind="ExternalInput").ap()
    ng_d = nc.dram_tensor("negg", [128, RT], f32, kind="ExternalInput").ap()
    c_d = nc.dram_tensor("cflat", [RT, 128, DJ], bf16, kind="ExternalInput").ap()
    cb_d = nc.dram_tensor("cbias", [128, RT * O], bf16, kind="ExternalInput").ap()
    on_d = nc.dram_tensor("onesd", [128, 2], bf16, kind="ExternalInput").ap()
    out_d = nc.dram_tensor("out", [NS, O], f32, kind="ExternalOutput").ap()

    def r32(ap):
        return ap if ap.dtype == f32r else ap.bitcast(f32r)

    with tile.TileContext(nc) as tc:
        from contextlib import ExitStack

        with ExitStack() as ctx:
            konst = ctx.enter_context(tc.tile_pool(name="konst", bufs=1))
            cw = ctx.enter_context(tc.tile_pool(name="cw", bufs=1))
            stp = ctx.enter_context(tc.tile_pool(name="stp", bufs=1))
            prodp = ctx.enter_context(tc.tile_pool(name="prodp", bufs=3))
            small = ctx.enter_context(tc.tile_pool(name="small", bufs=3))
            psum = ctx.enter_context(tc.tile_pool(name="psum", bufs=2, space="PSUM"))

            # ---- parameter / input loads
            a_sb = konst.tile([D, R], f32r)
            nc.sync.dma_start(a_sb[:], a_d.bitcast(f32r))
            b_sb = konst.tile([D, R], f32r)
            nc.sync.dma_start(b_sb[:], b_d.bitcast(f32r))
            ng_sb = konst.tile([128, RT], f32)
            nc.sync.dma_start(ng_sb[:], ng_d)
            xt_sb = konst.tile([D, NS], f32r)
            nc.sync.dma_start(xt_sb[:], xt_d.bitcast(f32r))
            ones = konst.tile([128, 2], bf16)
            nc.sync.dma_start(ones[:], on_d)

            # x^2 transposed (written rounded-to-fp32r for the PE)
            x2t = konst.tile([D, NS], f32r)
            nc.vector.tensor_tensor(
                x2t[:], xt_sb[:].bitcast(f32), xt_sb[:].bitcast(f32), ALU.mult
            )

            # ---- strengths^T: [r-tile partitions, n free], bf16
            st_tiles = []
            for rt in range(RT):
                sps = psum.tile([128, NS], f32, tag="bank", name=f"sps{rt}")  # 1 bank
                nc.tensor.matmul(
                    sps[:], r32(a_sb[:, ts(rt, 128)]), r32(x2t[:]),
                    start=True, stop=False,
                )
                nc.tensor.matmul(
                    sps[:], r32(b_sb[:, ts(rt, 128)]), r32(xt_sb[:]),
                    start=False, stop=True,
                )
                st = stp.tile([128, NS], bf16, name=f"st{rt}")
                nc.scalar.activation(
                    st[:], sps[:], AF.Exp, bias=ng_sb[:, rt : rt + 1], scale=-1.0
                )
                st_tiles.append(st)

            # ---- big weights
            cb_sb = cw.tile([128, RT * O], bf16)
            nc.sync.dma_start(cb_sb[:], cb_d)
            xn_sb = konst.tile([128, NT * D], bf16)
            nc.sync.dma_start(xn_sb[:], xn_d)
            c_sb = cw.tile([128, RT * DJ], bf16)
            for c in range(NCHUNK):
                for rt in range(RT):
                    eng = nc.sync if (c % 2 == 0) else nc.gpsimd
                    eng.dma_start(
                        c_sb[:, rt * DJ + c * CHUNK : rt * DJ + (c + 1) * CHUNK],
                        c_d[rt, :, ts(c, CHUNK)],
                    )

            # ---- per n-tile pipeline
            for nt in range(NT):
                st_n = [st[:, ts(nt, 128)] for st in st_tiles]

                # den column = sum_r strengths
                dps = psum.tile([128, NS], f32, tag="bank", name=f"dps{nt}")
                nc.tensor.matmul(
                    dps[:, :2], st_n[0], ones[:], start=True, stop=False
                )
                nc.tensor.matmul(
                    dps[:, :2], st_n[1], ones[:], start=False, stop=True
                )
                denc = small.tile([128, 1], f32, name=f"denc{nt}")
                nc.vector.tensor_scalar_add(denc[:], dps[:, :1], 1e-8)
                scalec = small.tile([128, 1], f32, name=f"scalec{nt}")
                nc.vector.reciprocal(scalec[:], denc[:])

                # bias consequent Tb
                bps = psum.tile([128, NS], f32, tag="bank", name=f"bps{nt}")
                nc.tensor.matmul(
                    bps[:, :O], st_n[0], cb_sb[:, 0:O], start=True, stop=False
                )
                nc.tensor.matmul(
                    bps[:, :O], st_n[1], cb_sb[:, O : 2 * O], start=False, stop=True
                )
                tb_sb = small.tile([128, O], f32, name=f"tb{nt}", tag="tb")
                nc.scalar.activation(tb_sb[:], bps[:, :O], AF.Copy)

                # prod layout: [n, j, d] (d contiguous)
                prod = prodp.tile([128, O, D], bf16, name=f"prod{nt}", tag="prod")

                xrow = xn_sb[:, ts(nt, D)]  # [128 n, 128 d] bf16
                xb = xrow.unsqueeze(1).broadcast_to([128, JPC, D])
                for c in range(NCHUNK):
                    tps = psum.tile(
                        [128, CHUNK], f32, tag="bank2", name=f"tps{nt}_{c}", bufs=3
                    )
                    for half in range(CHUNK // MM):
                        hsl = slice(half * MM, (half + 1) * MM)
                        base = c * CHUNK + half * MM
                        nc.tensor.matmul(
                            tps[:, hsl], st_n[0],
                            c_sb[:, 0 * DJ + base : 0 * DJ + base + MM],
                            start=True, stop=False,
                        )
                        nc.tensor.matmul(
                            tps[:, hsl], st_n[1],
                            c_sb[:, 1 * DJ + base : 1 * DJ + base + MM],
                            start=False, stop=True,
                        )
                    tview = tps[:].rearrange("p (j d) -> p j d", j=JPC)
                    oview = prod[:, c * JPC : (c + 1) * JPC, :]  # [128, 8, 128]
                    if c < FP32_DIRECT:
                        nc.vector.tensor_tensor(oview, tview, xb, ALU.mult)
                    else:
                        tcp = small.tile(
                            [128, JPC, D], bf16, tag="tcp", name=f"tcp{nt}_{c}",
                            bufs=4,
                        )
                        nc.scalar.activation(tcp[:], tps[:], AF.Copy)
                        nc.vector.tensor_tensor(oview, tcp[:], xb, ALU.mult)

                # tree-reduction over d (bf16, contiguous innermost)
                sbuf_s = small.tile([128, O, D // 2], bf16, tag="tree", name=f"s{nt}")
                nc.vector.tensor_tensor(
                    sbuf_s[:], prod[:, :, 0 : D // 2], prod[:, :, D // 2 : D], ALU.add
                )
                h = D // 2
                while h > 1:
                    h //= 2
                    nc.vector.tensor_tensor(
                        sbuf_s[:, :, 0:h], sbuf_s[:, :, 0:h],
                        sbuf_s[:, :, h : 2 * h], ALU.add,
                    )

                # acc = tree + Tb  (fused, reads Tb straight from PSUM)
                acc = small.tile([128, O], f32, name=f"acc{nt}")
                nc.vector.scalar_tensor_tensor(
                    acc[:], sbuf_s[:, :, 0], 1.0, tb_sb[:], ALU.mult, ALU.add
                )

                # softmax over j of logits = acc/den, fused:
                # exp(acc*scalec - max(acc)*scalec), max taken on unscaled acc
                negm = small.tile([128, 1], f32, name=f"negm{nt}")
                nc.vector.tensor_reduce(
                    negm[:], acc[:], axis=mybir.AxisListType.X, op=ALU.max,
                    negate=True,
                )
                negmb = small.tile([128, 1], f32, name=f"negmb{nt}")
                nc.vector.tensor_tensor(negmb[:], negm[:], scalec[:], ALU.mult)
                exps = small.tile([128, O], f32, name=f"exps{nt}")
                sume = small.tile([128, 1], f32, name=f"sume{nt}")
                nc.scalar.activation(
                    exps[:], acc[:], AF.Exp, bias=negmb[:], scale=scalec[:],
                    accum_out=sume[:],
                )
                rs = small.tile([128, 1], f32, name=f"rs{nt}")
                nc.vector.reciprocal(rs[:], sume[:])
                osb = small.tile([128, O], f32, name=f"osb{nt}")
                nc.scalar.activation(osb[:], exps[:], AF.Copy, scale=rs[:])
                nc.sync.dma_start(out_d[ts(nt, 128), :], osb[:])

    nc.compile()
    return nc


def _prep_inputs(X, centers, sigmas, coeffs):
    """Host-side sharding + layout transforms (numpy only)."""
    X = np.ascontiguousarray(X, dtype=np.float32)
    centers = np.asarray(centers, dtype=np.float32)
    sigmas = np.asarray(sigmas, dtype=np.float32)
    coeffs = np.asarray(coeffs, dtype=np.float32)

    inv2s2 = 1.0 / (2.0 * sigmas * sigmas)            # [R, D]
    A = np.ascontiguousarray(inv2s2.T)                # [D, R]
    B = np.ascontiguousarray((-centers / (sigmas * sigmas)).T)  # [D, R]
    G = (centers * centers * inv2s2).sum(axis=1)      # [R]
    negG = np.ascontiguousarray(-G.reshape(RT, 128).T)  # [128, RT]

    # C in [r, (j, d)] layout, bf16
    Cjd = np.ascontiguousarray(coeffs[:, :D, :].transpose(0, 2, 1))  # [R, O, D]
    Ck = np.ascontiguousarray(Cjd.reshape(RT, 128, DJ).astype(BF16))
    Cb = np.ascontiguousarray(
        coeffs[:, D, :].reshape(RT, 128, O).transpose(1, 0, 2).reshape(128, RT * O)
    ).astype(BF16)

    in_maps = []
    for i in range(NCORES):
        Xs = X[i * NS : (i + 1) * NS]                  # [512, 128]
        xt = np.ascontiguousarray(Xs.T)                # [128, 512]
        xn = np.ascontiguousarray(
            Xs.reshape(NT, 128, D).transpose(1, 0, 2).reshape(128, NT * D)
        ).astype(BF16)
        in_maps.append(
            {
                "xt": xt,
                "xn": xn,
                "a_p": A,
                "b_p": B,
                "negg": negG,
                "cflat": Ck,
                "cbias": Cb,
                "onesd": np.ones((128, 2), dtype=BF16),
            }
        )
    return in_maps


def kernel(X, centers, sigmas, coeffs):
    from concourse.bass_utils import run_bass_kernel_spmd

    if "nc" not in _CACHE:
        _CACHE["nc"] = _build()
    nc = _CACHE["nc"]

    in_maps = _prep_inputs(X, centers, sigmas, coeffs)
    res = run_bass_kernel_spmd(nc, in_maps, list(range(NCORES)))
    out = np.concatenate([res.results[i]["out"] for i in range(NCORES)], axis=0)
    return out.astype(np.float32)


if __name__ == "__main__":
    rng = np.random.default_rng(0)
    X = rng.standard_normal((N, D), dtype=np.float32)
    centers = 0.5 * rng.standard_normal((R, D)).astype(np.float32)
    sigmas = (1.5 + rng.random((R, D))).astype(np.float32)
    coeffs = (0.02 * rng.standard_normal((R, D + 1, O))).astype(np.float32)
    out = kernel(X=X, centers=centers, sigmas=sigmas, coeffs=coeffs)
    print(out.shape, out.dtype, out.sum(axis=1)[:4])


# revision 10
# speedup vs baseline: 1.3512x; 1.0125x over previous
"""Trainium2 Bass kernel for nn_CustomANFIS (N=4096, D=128, R=256, O=64).

Math (reference):
  memb[n,r,d]  = exp(-(x[n,d]-c[r,d])^2 / (2 s[r,d]^2))
  str[n,r]     = prod_d memb = exp(-q[n,r]) with
                 q[n,r] = sum_d x^2[n,d]*A[d,r] + sum_d x[n,d]*B[d,r] + G[r],
                 A = 1/(2 s^2), B = -c/s^2, G = sum_d c^2/(2 s^2)
  den[n]       = sum_r str + 1e-8
  W[n,r,:]     = x[n,:] @ coeffs[r,:D,:] + coeffs[r,D,:]
  out          = softmax_j( (1/den) * sum_r str[n,r] * W[n,r,j] )

Device algorithm (data-parallel over N across 8 cores):
  1. strengths^T [r (2 part-tiles), n=512] via 2 fp32r accumulating matmuls
     + ACT exp (per-partition bias=-G), written as bf16.
  2. den column per n-tile via matmul(lhsT = sT-slice, rhs = ones).
  3. T[n, (j,d)] = sum_r sT[r,n] * C[r, (j,d)] in bf16 (16 chunks of 512 =
     4 j x 128 d per n-tile, PSUM-accumulated over the 2 r K-tiles), plus
     Tb[n,j] = sum_r sT[r,n]*Cb[r,j].
  4. prod[n, j, d] = X[n,d] * T[n,j,d]: ACT casts PSUM->SBUF bf16, DVE
     multiplies at 2x (or reads PSUM fp32 directly for some chunks, GPSIMD
     takes a share); then a bf16 tree-reduction over d and a fused
     (tree + Tb) add -> acc[n,j].
  5. logits = acc/den; softmax over j via ACT exp + accum_out.
"""

import numpy as np
import ml_dtypes

N, D, R, O = 4096, 128, 256, 64
NCORES = 8
NS = N // NCORES          # 512 rows per core
NT = NS // 128            # 4 n-tiles per core
RT = R // 128             # 2 r k-tiles
DJ = D * O                # 8192
CHUNK = 1024              # 2 PSUM banks per chunk
NCHUNK = DJ // CHUNK      # 8 chunks (8 j x 128 d each)
JPC = CHUNK // D          # 8 j per chunk
MM = 512                  # moving free dim per matmul

# per n-tile chunk split: the first FP32_DIRECT chunks are multiplied by DVE
# straight from PSUM (fp32, 1x); the rest are ACT-cast to bf16 SBUF and
# multiplied by DVE at 2x, except the last GPS_CHUNKS which go to GPSIMD.
FP32_DIRECT = 1
GPS_CHUNKS = 0

_CACHE = {}
BF16 = ml_dtypes.bfloat16


def _build():
    import concourse.bass as bass
    import concourse.tile as tile
    from concourse import bacc, mybir

    f32 = mybir.dt.float32
    f32r = mybir.dt.float32r
    bf16 = mybir.dt.bfloat16
    AF = mybir.ActivationFunctionType
    ALU = mybir.AluOpType
    ts = bass.ts

    nc = bacc.Bacc(
        "TRN2", target_bir_lowering=False, debug=False, num_devices=NCORES
    )

    xt_d = nc.dram_tensor("xt", [D, NS], f32, kind="ExternalInput").ap()
    xn_d = nc.dram_tensor("xn", [128, NT * D], bf16, kind="ExternalInput").ap()
    a_d = nc.dram_tensor("a_p", [D, R], f32, kind="ExternalInput").ap()
    b_d = nc.dram_tensor("b_p", [D, R], f32, kind="ExternalInput").ap()
    ng_d = nc.dram_tensor("negg", [128, RT], f32, kind="ExternalInput").ap()
    c_d = nc.dram_tensor("cflat", [RT, 128, DJ], bf16, kind="ExternalInput").ap()
    cbo_d = nc.dram_tensor("cbo", [128, RT * (O + 2)], bf16, kind="ExternalInput").ap()
    out_d = nc.dram_tensor("out", [NS, O], f32, kind="ExternalOutput").ap()

    def r32(ap):
        return ap if ap.dtype == f32r else ap.bitcast(f32r)

    with tile.TileContext(nc) as tc:
        from contextlib import ExitStack

        with ExitStack() as ctx:
            konst = ctx.enter_context(tc.tile_pool(name="konst", bufs=1))
            cw = ctx.enter_context(tc.tile_pool(name="cw", bufs=1))
            stp = ctx.enter_context(tc.tile_pool(name="stp", bufs=1))
            prodp = ctx.enter_context(tc.tile_pool(name="prodp", bufs=3))
            small = ctx.enter_context(tc.tile_pool(name="small", bufs=3))
            psum = ctx.enter_context(tc.tile_pool(name="psum", bufs=2, space="PSUM"))

            # ---- parameter / input loads
            a_sb = konst.tile([D, R], f32r)
            nc.sync.dma_start(a_sb[:, 0:128], a_d[:, 0:128].bitcast(f32r))
            nc.gpsimd.dma_start(a_sb[:, 128:256], a_d[:, 128:256].bitcast(f32r))
            b_sb = konst.tile([D, R], f32r)
            nc.sync.dma_start(b_sb[:, 0:128], b_d[:, 0:128].bitcast(f32r))
            nc.gpsimd.dma_start(b_sb[:, 128:256], b_d[:, 128:256].bitcast(f32r))
            ng_sb = konst.tile([128, RT], f32)
            nc.sync.dma_start(ng_sb[:], ng_d)
            xt_sb = konst.tile([D, NS], f32r)
            for q in range(4):
                eng = nc.sync if q % 2 == 0 else nc.gpsimd
                eng.dma_start(
                    xt_sb[:, q * 128 : (q + 1) * 128],
                    xt_d[:, q * 128 : (q + 1) * 128].bitcast(f32r),
                )


            # x^2 transposed (written rounded-to-fp32r for the PE)
            x2t = konst.tile([D, NS], f32r)
            nc.vector.tensor_tensor(
                x2t[:], xt_sb[:].bitcast(f32), xt_sb[:].bitcast(f32), ALU.mult
            )

            # ---- strengths^T: [r-tile partitions, n free], bf16
            st_tiles = []
            for rt in range(RT):
                sps = psum.tile([128, CHUNK], f32, tag="bank2", name=f"sps{rt}", bufs=4)
                nc.tensor.matmul(
                    sps[:, :NS], r32(a_sb[:, ts(rt, 128)]), r32(x2t[:]),
                    start=True, stop=False,
                )
                nc.tensor.matmul(
                    sps[:, :NS], r32(b_sb[:, ts(rt, 128)]), r32(xt_sb[:]),
                    start=False, stop=True,
                )
                st = stp.tile([128, NS], bf16, name=f"st{rt}")
                nc.scalar.activation(
                    st[:], sps[:, :NS], AF.Exp, bias=ng_sb[:, rt : rt + 1], scale=-1.0
                )
                st_tiles.append(st)

            # ---- big weights
            cbo_sb = cw.tile([128, RT * (O + 2)], bf16)
            nc.sync.dma_start(cbo_sb[:], cbo_d)
            xn_sb = konst.tile([128, NT * D], bf16)
            nc.sync.dma_start(xn_sb[:], xn_d)
            c_sb = cw.tile([128, RT * DJ], bf16)
            qi = 0
            for c in range(NCHUNK):
                for half in range(CHUNK // MM):
                    for rt in range(RT):
                        base = c * CHUNK + half * MM
                        eng = nc.sync if qi % 2 == 0 else nc.gpsimd
                        qi += 1
                        eng.dma_start(
                            c_sb[:, rt * DJ + base : rt * DJ + base + MM],
                            c_d[rt, :, base : base + MM],
                        )

            # ---- den + bias consequent for all n-tiles (merged rhs)
            scalecs, tbs = [], []
            for nt in range(NT):
                st_n = [st[:, ts(nt, 128)] for st in st_tiles]
                dbp = psum.tile([128, CHUNK], f32, tag="bank2", name=f"dbp{nt}", bufs=4)
                nc.tensor.matmul(
                    dbp[:, : O + 2], st_n[0], cbo_sb[:, 0 : O + 2],
                    start=True, stop=False,
                )
                nc.tensor.matmul(
                    dbp[:, : O + 2], st_n[1], cbo_sb[:, O + 2 : 2 * (O + 2)],
                    start=False, stop=True,
                )
                denc = small.tile([128, 1], f32, name=f"denc{nt}")
                nc.vector.tensor_scalar_add(denc[:], dbp[:, :1], 1e-8)
                scalec = small.tile([128, 1], f32, name=f"scalec{nt}")
                nc.vector.reciprocal(scalec[:], denc[:])
                scalecs.append(scalec)
                tb_sb = small.tile([128, O], f32, name=f"tb{nt}", tag="tb")
                nc.scalar.activation(tb_sb[:], dbp[:, 2 : O + 2], AF.Copy)
                tbs.append(tb_sb)

            # ---- per n-tile pipeline
            for nt in range(NT):
                st_n = [st[:, ts(nt, 128)] for st in st_tiles]
                scalec = scalecs[nt]
                tb_sb = tbs[nt]

                # prod layout: [n, j, d] (d contiguous)
                prod = prodp.tile([128, O, D], bf16, name=f"prod{nt}", tag="prod")

                xrow = xn_sb[:, ts(nt, D)]  # [128 n, 128 d] bf16
                xb = xrow.unsqueeze(1).broadcast_to([128, JPC, D])
                for c in range(NCHUNK):
                    tps = psum.tile(
                        [128, CHUNK], f32, tag="bank2", name=f"tps{nt}_{c}", bufs=4
                    )
                    for half in range(CHUNK // MM):
                        hsl = slice(half * MM, (half + 1) * MM)
                        base = c * CHUNK + half * MM
                        nc.tensor.matmul(
                            tps[:, hsl], st_n[0],
                            c_sb[:, 0 * DJ + base : 0 * DJ + base + MM],
                            start=True, stop=False,
                        )
                        nc.tensor.matmul(
                            tps[:, hsl], st_n[1],
                            c_sb[:, 1 * DJ + base : 1 * DJ + base + MM],
                            start=False, stop=True,
                        )
                    tview = tps[:].rearrange("p (j d) -> p j d", j=JPC)
                    oview = prod[:, c * JPC : (c + 1) * JPC, :]  # [128, 8, 128]
                    if c < FP32_DIRECT:
                        nc.vector.tensor_tensor(oview, tview, xb, ALU.mult)
                    else:
                        tcp = small.tile(
                            [128, JPC, D], bf16, tag="tcp", name=f"tcp{nt}_{c}",
                            bufs=4,
                        )
                        nc.scalar.activation(tcp[:], tps[:], AF.Copy)
                        nc.vector.tensor_tensor(oview, tcp[:], xb, ALU.mult)

                # tree-reduction over d (bf16, contiguous innermost)
                sbuf_s = small.tile([128, O, D // 2], bf16, tag="tree", name=f"s{nt}")
                nc.vector.tensor_tensor(
                    sbuf_s[:], prod[:, :, 0 : D // 2], prod[:, :, D // 2 : D], ALU.add
                )
                h = D // 2
                while h > 1:
                    h //= 2
                    nc.vector.tensor_tensor(
                        sbuf_s[:, :, 0:h], sbuf_s[:, :, 0:h],
                        sbuf_s[:, :, h : 2 * h], ALU.add,
                    )

                # acc = tree + Tb  (fused, reads Tb straight from PSUM)
                acc = small.tile([128, O], f32, name=f"acc{nt}")
                nc.vector.scalar_tensor_tensor(
                    acc[:], sbuf_s[:, :, 0], 1.0, tb_sb[:], ALU.mult, ALU.add
                )

                # softmax over j of logits = acc/den, fused:
                # exp(acc*scalec - max(acc)*scalec), max taken on unscaled acc
                negm = small.tile([128, 1], f32, name=f"negm{nt}")
                nc.vector.tensor_reduce(
                    negm[:], acc[:], axis=mybir.AxisListType.X, op=ALU.max,
                    negate=True,
                )
                negmb = small.tile([128, 1], f32, name=f"negmb{nt}")
                nc.vector.tensor_tensor(negmb[:], negm[:], scalec[:], ALU.mult)
                exps = small.tile([128, O], f32, name=f"exps{nt}")
                sume = small.tile([128, 1], f32, name=f"sume{nt}")
                nc.scalar.activation(
                    exps[:], acc[:], AF.Exp, bias=negmb[:], scale=scalec[:],
                    accum_out=sume[:],
                )
                rs = small.tile([128, 1], f32, name=f"rs{nt}")
                nc.vector.reciprocal(rs[:], sume[:])
                osb = small.tile([128, O], f32, name=f"osb{nt}")
                nc.scalar.activation(osb[:], exps[:], AF.Copy, scale=rs[:])
                nc.sync.dma_start(out_d[ts(nt, 128), :], osb[:])

    nc.compile()
    return nc


def _prep_inputs(X, centers, sigmas, coeffs):
    """Host-side sharding + layout transforms (numpy only)."""
    X = np.ascontiguousarray(X, dtype=np.float32)
    centers = np.asarray(centers, dtype=np.float32)
    sigmas = np.asarray(sigmas, dtype=np.float32)
    coeffs = np.asarray(coeffs, dtype=np.float32)

    inv2s2 = 1.0 / (2.0 * sigmas * sigmas)            # [R, D]
    A = np.ascontiguousarray(inv2s2.T)                # [D, R]
    B = np.ascontiguousarray((-centers / (sigmas * sigmas)).T)  # [D, R]
    G = (centers * centers * inv2s2).sum(axis=1)      # [R]
    negG = np.ascontiguousarray(-G.reshape(RT, 128).T)  # [128, RT]

    # C in [r, (j, d)] layout, bf16
    Cjd = np.ascontiguousarray(coeffs[:, :D, :].transpose(0, 2, 1))  # [R, O, D]
    Ck = np.ascontiguousarray(Cjd.reshape(RT, 128, DJ).astype(BF16))
    Cb = coeffs[:, D, :].reshape(RT, 128, O).transpose(1, 0, 2)  # [128, RT, O]
    Cbo = np.ones((128, RT, O + 2), dtype=np.float32)
    Cbo[:, :, 2:] = Cb
    Cbo = np.ascontiguousarray(Cbo.reshape(128, RT * (O + 2))).astype(BF16)

    in_maps = []
    for i in range(NCORES):
        Xs = X[i * NS : (i + 1) * NS]                  # [512, 128]
        xt = np.ascontiguousarray(Xs.T)                # [128, 512]
        xn = np.ascontiguousarray(
            Xs.reshape(NT, 128, D).transpose(1, 0, 2).reshape(128, NT * D)
        ).astype(BF16)
        in_maps.append(
            {
                "xt": xt,
                "xn": xn,
                "a_p": A,
                "b_p": B,
                "negg": negG,
                "cflat": Ck,
                "cbo": Cbo,
            }
        )
    return in_maps


def kernel(X, centers, sigmas, coeffs):
    from concourse.bass_utils import run_bass_kernel_spmd

    if "nc" not in _CACHE:
        _CACHE["nc"] = _build()
    nc = _CACHE["nc"]

    in_maps = _prep_inputs(X, centers, sigmas, coeffs)
    res = run_bass_kernel_spmd(nc, in_maps, list(range(NCORES)))
    out = np.concatenate([res.results[i]["out"] for i in range(NCORES)], axis=0)
    return out.astype(np.float32)


if __name__ == "__main__":
    rng = np.random.default_rng(0)
    X = rng.standard_normal((N, D), dtype=np.float32)
    centers = 0.5 * rng.standard_normal((R, D)).astype(np.float32)
    sigmas = (1.5 + rng.random((R, D))).astype(np.float32)
    coeffs = (0.02 * rng.standard_normal((R, D + 1, O))).astype(np.float32)
    out = kernel(X=X, centers=centers, sigmas=sigmas, coeffs=coeffs)
    print(out.shape, out.dtype, out.sum(axis=1)[:4])
